# revision 6
# baseline (speedup 1.0000x reference)
"""KronEmbedding lookup kernel for 8 TRN2 NeuronCores.

Math: w = einsum('sia,sjb->ijab', A, B).reshape(50176, 2048); out = w[x].
Never materializes w. Per token t with i=x//224, j=x%224:
    out[t] = sum_s outer(A[s,i,:], B[s,j,:])   -> (64*32 = 2048 floats)

Strategy (data-parallel over tokens, 1024 tokens/core, all bf16 on the wire):
- Tokens are processed in 64 groups of 16 (k in [0,16), khalf=k//8, kk=k%8).
  Contraction partition p = 8k + s (s in [0,8)).
- ONE K=128 matmul per group: out[(khalf,a), (k8,b)] =
    sum_p T[p, (khalf,a)] * BD2[p, (k8,b)]
  T (stationary) is block-diagonal over khalf: T[p<64, half=1]=0,
  T[p>=64, half=0]=0 - zeros shipped from HBM inside TA (1 MiB, one DMA).
  BD2 (moving) is block-diagonal over kk: only rows with p//8 in {k8, k8+8}
  are nonzero for output column block k8. Zeros are memset on-device
  (8 column-block memsets split across DVE/ACT/Pool); the compact B data
  (0.5 MiB) is scattered into the diagonal blocks by 8 DMAs whose elements
  are 4 KB contiguous per partition (full DMA bus rate).
- Per 16-token group the PE streams only 256 columns (16 cols/token), half
  of the previous two-quadrant scheme.
- PSUM -> bf16 SBUF evacuation alternates DVE/ACT; 8x 512KB result DMAs.
- Host: upcast bf16 -> fp32 and reorder to token-major (untimed).
"""
import numpy as np
import ml_dtypes
from contextlib import ExitStack

import concourse.bass as bass
import concourse.bacc as bacc
import concourse.tile as tile
import concourse.mybir as mybir
from concourse import bass_utils

dt = mybir.dt
BF16 = ml_dtypes.bfloat16

R, M1, N1, M2, N2 = 8, 224, 64, 224, 32
VOCAB, EMB = M1 * M2, N1 * N2          # 50176, 2048
BATCH, SEQ = 4, 2048
NTOK = BATCH * SEQ                     # 8192
NCORES = 8
TPC = NTOK // NCORES                   # 1024 tokens per core
NGRP = TPC // 16                       # 64 groups of 16 tokens
NWARM = 8

_CACHE = {}


def _build():
    nc = bacc.Bacc("TRN2", num_devices=NCORES)
    TA = nc.dram_tensor("TA", [128, 2, 64, NGRP], dt.bfloat16, kind="ExternalInput")
    GBS = nc.dram_tensor("GBS", [8, 2, 8, 32, NGRP], dt.bfloat16, kind="ExternalInput")
    out = nc.dram_tensor("out", [8, 128, 2048], dt.bfloat16, kind="ExternalOutput")

    with tile.TileContext(nc) as tc, ExitStack() as ctx:
        const_pool = ctx.enter_context(tc.tile_pool(name="const", bufs=1))
        ev_pool = ctx.enter_context(tc.tile_pool(name="ev", bufs=3))
        ps_pool = ctx.enter_context(tc.tile_pool(name="ps", bufs=3, space="PSUM"))
        wps_pool = ctx.enter_context(tc.tile_pool(name="wps", bufs=1, space="PSUM"))

        # PE warmup: dependency-free matmuls release the HAM clock gate while
        # the BD2 zero-blocks are being memset.
        warm = const_pool.tile([128, 512], dt.bfloat16, tag="warm")
        nc.vector.memset(warm[:], 0.0)
        wps = wps_pool.tile([128, 512], dt.float32, tag="wps")
        for _ in range(NWARM):
            nc.tensor.matmul(wps[:], warm[:, 0:128], warm[:], start=True, stop=True)

        # Stationary operand, zeros included (shipped from HBM).
        T = const_pool.tile([128, 2, 64, NGRP], dt.bfloat16, tag="T", name="T")
        nc.sync.dma_start(T[:], TA[:])

        # Moving operand: [p, k8, g, b]. Zero the 8 column blocks on
        # DVE (fast memset) / ACT / Pool, then scatter the compact B rows
        # into the diagonal blocks (partitions {8*kk+s} u {64+8*kk+s}).
        BD2 = const_pool.tile([128, 8, 32, NGRP], dt.bfloat16, tag="BD2", name="BD2")
        def ms(engine, ap):
            if engine is nc.scalar:
                engine.memzero(ap)
            else:
                engine.memset(ap, 0.0)

        ms_engine = {
            0: nc.scalar, 1: nc.gpsimd, 2: nc.scalar, 3: nc.gpsimd,
            4: nc.vector, 5: nc.vector, 6: nc.vector, 7: nc.vector,
        }
        # memset issue order per engine: DVE does 6,7 first so the Pool-queue
        # SWDGE scatters (blocks 6,7) can start generating early.
        for kk in (0, 1, 2, 3, 6, 7, 4, 5):
            ms(ms_engine[kk], BD2[:, kk])

        sc_engine = {
            0: nc.sync, 1: nc.sync, 2: nc.scalar, 3: nc.scalar,
            4: nc.sync, 5: nc.scalar, 6: nc.gpsimd, 7: nc.gpsimd,
        }
        for kk in range(8):
            for xh in range(2):
                sc_engine[kk].dma_start(
                    BD2[64 * xh + 8 * kk:64 * xh + 8 * kk + 8, kk],
                    GBS[kk, xh],
                )

        # Main stream: 8 chunks x 8 groups. One K=128 matmul per group.
        for chunk in range(8):
            ev = ev_pool.tile([128, 2048], dt.bfloat16, tag="ev")
            for h2 in range(2):
                ps = ps_pool.tile([128, 1024], dt.float32, tag="ps")
                for h in range(4):
                    g = 8 * chunk + 4 * h2 + h
                    nc.tensor.matmul(
                        ps[:, 256 * h:256 * h + 256],
                        T[:, :, :, g],
                        BD2[:, :, :, g],
                        start=True,
                        stop=True,
                    )
                if h2 == 0:
                    nc.scalar.copy(ev[:, 0:1024], ps[:])
                else:
                    nc.vector.tensor_copy(ev[:, 1024:2048], ps[:])
            (nc.scalar if chunk % 2 == 0 else nc.sync).dma_start(out[chunk], ev[:])

    nc.compile()
    return nc


def kernel(A: np.ndarray, B: np.ndarray, x: np.ndarray) -> np.ndarray:
    Abf = np.asarray(A, dtype=np.float32).astype(BF16)    # [8, 224, 64]
    Bbf = np.asarray(B, dtype=np.float32).astype(BF16)    # [8, 224, 32]
    xl = np.asarray(x).astype(np.int64).reshape(-1)       # [8192]
    i_all = (xl // M2).astype(np.int64)
    j_all = (xl % M2).astype(np.int64)

    if "nc" not in _CACHE:
        _CACHE["nc"] = _build()
    nc = _CACHE["nc"]

    in_maps = []
    for c in range(NCORES):
        sl = slice(c * TPC, (c + 1) * TPC)
        IA = i_all[sl].reshape(NGRP, 16)                  # [g, k]
        JB = j_all[sl].reshape(NGRP, 16)

        # TA[p, half, g, a]: p = 8k+s. Lower half (p<64, k<8) holds data in
        # half=0; upper half (k>=8) in half=1; the other half is zeros.
        TA = np.zeros((128, 2, 64, NGRP), dtype=BF16)
        TA[0:64, 0] = np.ascontiguousarray(
            Abf[:, IA[:, :8], :].transpose(2, 0, 3, 1)    # [8k, 8s, a, g]
        ).reshape(64, 64, NGRP)
        TA[64:128, 1] = np.ascontiguousarray(
            Abf[:, IA[:, 8:], :].transpose(2, 0, 3, 1)
        ).reshape(64, 64, NGRP)

        # GBS[kk, x, s, b, g] = B[s, j_t, b] with token t = 16g + 8x + kk.
        GB = Bbf[:, JB, :]                                # [s, g, 16k, b]
        GB = GB.transpose(2, 0, 3, 1)                     # [16k, s, b, g]
        GBS = np.ascontiguousarray(
            GB.reshape(2, 8, 8, 32, NGRP).transpose(1, 0, 2, 3, 4)
        )                                                 # [kk, x, s, b, g]
        in_maps.append(dict(TA=TA, GBS=GBS))

    _CACHE["in_maps"] = in_maps
    res = bass_utils.run_bass_kernel_spmd(nc, in_maps, core_ids=list(range(NCORES)))

    outs = []
    for c in range(NCORES):
        o = np.asarray(res.results[c]["out"]).astype(np.float32)  # [8,128,2048]
        # rows p = 64*xhalf + a; cols = (gi, k8, b); g = 8*chunk + gi
        o = o.reshape(8, 2, 64, 8, 8, 32)                # [chunk, xh, a, gi, k8, b]
        # token t = 128*chunk + 16*gi + 8*xh + k8
        o = o.transpose(0, 3, 1, 4, 2, 5)                # [chunk, gi, xh, k8, a, b]
        outs.append(o.reshape(TPC, EMB))
    full = np.concatenate(outs, axis=0)                  # [8192, 2048]
    return full.reshape(BATCH, SEQ, EMB)


# revision 8
# speedup vs baseline: 1.3674x; 1.3674x over previous
"""KronEmbedding lookup kernel for 8 TRN2 NeuronCores.

Math: w = einsum('sia,sjb->ijab', A, B).reshape(50176, 2048); out = w[x].
Never materializes w. Per token t with i=x//224, j=x%224:
    out[t] = sum_s outer(A[s,i,:], B[s,j,:])   -> (64*32 = 2048 floats)

Strategy (data-parallel over tokens, 1024 tokens/core, all bf16 on the wire):
- Tokens in 64 groups of 16 (k in [0,16)); contraction partition p = 8k+s.
- Per group, two overlapping sub-array matmuls (tile_position (0,0)/(64,64)):
  contraction rows 64*hh..64*hh+64 (tokens k in [8hh, 8hh+8)) x stationary
  AG rows -> out partitions (hh, a); moving operand BD[64*hh.., g, :] is the
  block-diagonal ([256] = 8 tokens x 32) - both operands CONTIGUOUS per
  group, which is required for full-rate PE streaming (strided rhs is 4x
  slower on HW).
- BD is 8x zero-padded (4 MiB); shipping it fully costs 11.6us of DMA bus.
  Hybrid build instead:
    groups 0..15  loaded pre-padded from HBM (1 MiB) so matmuls start
                  immediately;
    groups 16..63 zero-filled on-device (memsets split across DVE/ACT/Pool,
                  running under the early matmul/out-DMA stream) and the
                  compact B rows (0.375 MiB) scattered into the diagonal
                  blocks by 16 DMAs.
- AG compact (1 MiB, no padding), 2 DMAs.
- PSUM -> bf16 SBUF evacuation split DVE/ACT/Pool; 8x 512KB result DMAs.
- Host: upcast bf16 -> fp32 and reorder to token-major (untimed).
"""
import numpy as np
import ml_dtypes
from contextlib import ExitStack

import concourse.bass as bass
import concourse.bacc as bacc
import concourse.tile as tile
import concourse.mybir as mybir
from concourse import bass_utils

dt = mybir.dt
BF16 = ml_dtypes.bfloat16

R, M1, N1, M2, N2 = 8, 224, 64, 224, 32
VOCAB, EMB = M1 * M2, N1 * N2          # 50176, 2048
BATCH, SEQ = 4, 2048
NTOK = BATCH * SEQ                     # 8192
NCORES = 8
TPC = NTOK // NCORES                   # 1024 tokens per core
NGRP = TPC // 16                       # 64 groups of 16 tokens
QP = 16                                # pre-padded leading groups
NREST = NGRP - QP                      # scatter-built groups
NWARM = 8

_CACHE = {}


def _build():
    nc = bacc.Bacc("TRN2", num_devices=NCORES)
    AG = nc.dram_tensor("AG", [128, NGRP, 64], dt.bfloat16, kind="ExternalInput")
    BDF = nc.dram_tensor("BDF", [128, QP, 256], dt.bfloat16, kind="ExternalInput")
    GBR = nc.dram_tensor("GBR", [8, 2, 8, NREST, 32], dt.bfloat16,
                         kind="ExternalInput")
    out = nc.dram_tensor("out", [8, 128, 2048], dt.bfloat16, kind="ExternalOutput")

    with tile.TileContext(nc) as tc, ExitStack() as ctx:
        const_pool = ctx.enter_context(tc.tile_pool(name="const", bufs=1))
        ev_pool = ctx.enter_context(tc.tile_pool(name="ev", bufs=3))
        ps_pool = ctx.enter_context(tc.tile_pool(name="ps", bufs=3, space="PSUM"))
        wps_pool = ctx.enter_context(tc.tile_pool(name="wps", bufs=1, space="PSUM"))

        # PE warmup: dependency-free matmuls release the HAM clock gate while
        # the input DMAs land.
        warm = const_pool.tile([128, 512], dt.bfloat16, tag="warm")
        nc.vector.memset(warm[:], 0.0)
        wps = wps_pool.tile([128, 512], dt.float32, tag="wps")
        for _ in range(NWARM):
            nc.tensor.matmul(wps[:], warm[:, 0:128], warm[:], start=True, stop=True)

        ag = const_pool.tile([128, NGRP, 64], dt.bfloat16, tag="ag", name="ag")
        bd = const_pool.tile([128, NGRP, 256], dt.bfloat16, tag="bd", name="bd")

        # Input loads, earliest (these drain during the runtime preamble).
        nc.sync.dma_start(ag[:, 0:32], AG[:, 0:32])
        nc.sync.dma_start(bd[:, 0:QP], BDF[:])
        nc.scalar.dma_start(ag[:, 32:NGRP], AG[:, 32:NGRP])

        # Zero-fill groups QP.. (column-block split across engines), then
        # scatter the compact B rows into the diagonal 64B blocks.
        rg = [(QP + NREST * i // 8, QP + NREST * (i + 1) // 8) for i in range(8)]
        ms_engine = [nc.vector, nc.vector, nc.vector, nc.scalar,
                     nc.scalar, nc.gpsimd, nc.gpsimd, nc.gpsimd]
        for (g0, g1), eng in zip(rg, ms_engine):
            if eng is nc.scalar:
                eng.memzero(bd[:, g0:g1])
            else:
                eng.memset(bd[:, g0:g1], 0.0)

        for kk in range(8):
            for xh in range(2):
                (nc.sync if kk < 4 else nc.scalar).dma_start(
                    bd[64 * xh + 8 * kk:64 * xh + 8 * kk + 8,
                       QP:NGRP, 32 * kk:32 * kk + 32],
                    GBR[kk, xh],
                )

        # Main stream: 8 chunks x 8 groups; 2 sub-array matmuls per group.
        ev_engine = [nc.scalar, nc.vector, nc.scalar, nc.vector,
                     nc.scalar, nc.vector, nc.scalar, nc.vector,
                     nc.scalar, nc.vector, nc.scalar, nc.vector,
                     nc.scalar, nc.vector, nc.scalar, nc.vector]
        for chunk in range(8):
            ev = ev_pool.tile([128, 2048], dt.bfloat16, tag="ev")
            for h2 in range(2):
                ps = ps_pool.tile([128, 1024], dt.float32, tag="ps")
                for h in range(4):
                    g = 8 * chunk + 4 * h2 + h
                    for hh in range(2):
                        nc.tensor.matmul(
                            ps[64 * hh:64 * hh + 64, 256 * h:256 * h + 256],
                            ag[64 * hh:64 * hh + 64, g, :],
                            bd[64 * hh:64 * hh + 64, g, :],
                            start=True,
                            stop=True,
                            tile_position=(64 * hh, 64 * hh),
                        )
                eng = ev_engine[2 * chunk + h2]
                if eng is nc.vector:
                    eng.tensor_copy(ev[:, 1024 * h2:1024 * h2 + 1024], ps[:])
                elif eng is nc.scalar:
                    eng.copy(ev[:, 1024 * h2:1024 * h2 + 1024], ps[:])
                else:
                    eng.tensor_copy(ev[:, 1024 * h2:1024 * h2 + 1024], ps[:])
            (nc.scalar if chunk % 2 == 0 else nc.sync).dma_start(out[chunk], ev[:])

    nc.compile()
    return nc


def kernel(A: np.ndarray, B: np.ndarray, x: np.ndarray) -> np.ndarray:
    Abf = np.asarray(A, dtype=np.float32).astype(BF16)    # [8, 224, 64]
    Bbf = np.asarray(B, dtype=np.float32).astype(BF16)    # [8, 224, 32]
    xl = np.asarray(x).astype(np.int64).reshape(-1)       # [8192]
    i_all = (xl // M2).astype(np.int64)
    j_all = (xl % M2).astype(np.int64)

    if "nc" not in _CACHE:
        _CACHE["nc"] = _build()
    nc = _CACHE["nc"]

    in_maps = []
    for c in range(NCORES):
        sl = slice(c * TPC, (c + 1) * TPC)
        IA = i_all[sl].reshape(NGRP, 16)                  # [g, k]
        JB = j_all[sl].reshape(NGRP, 16)

        # AG[p, g, a] = A[s, i_t, a], p = 8k+s, t = 16g+k (compact lhsT)
        AGh = np.ascontiguousarray(
            Abf[:, IA, :].transpose(2, 0, 1, 3)           # [16k, 8s, g, a]
        ).reshape(128, NGRP, 64)

        # GB[k, s, g, b] = B[s, j_t, b]
        GB = Bbf[:, JB, :].transpose(2, 0, 1, 3)          # [16k, 8s, g, b]

        # BDF: pre-padded block-diagonal for the first QP groups.
        BDFh = np.zeros((16, 8, QP, 8, 32), dtype=BF16)   # [k, s, g, k8, b]
        for k in range(16):
            BDFh[k, :, :, k % 8, :] = GB[k, :, 0:QP, :]
        BDFh = BDFh.reshape(128, QP, 256)

        # GBR[kk, xh, s, grest, b] = B row of token k = 8*xh + kk.
        GBRh = np.ascontiguousarray(
            GB[:, :, QP:NGRP, :].reshape(2, 8, 8, NREST, 32).transpose(1, 0, 2, 3, 4)
        )
        in_maps.append(dict(AG=AGh, BDF=BDFh, GBR=GBRh))

    _CACHE["in_maps"] = in_maps
    res = bass_utils.run_bass_kernel_spmd(nc, in_maps, core_ids=list(range(NCORES)))

    outs = []
    for c in range(NCORES):
        o = np.asarray(res.results[c]["out"]).astype(np.float32)  # [8,128,2048]
        # rows: (hh, a); cols within chunk: (h2, h, k8, b), g = 8*chunk+4*h2+h
        o = o.reshape(8, 2, 64, 2, 4, 8, 32)             # [chunk, hh, a, h2, h, k8, b]
        # token t = 16*g + 8*hh + k8 = 128*chunk + 16*(4*h2+h) + 8*hh + k8
        o = o.transpose(0, 3, 4, 1, 5, 2, 6)             # [chunk, h2, h, hh, k8, a, b]
        outs.append(o.reshape(TPC, EMB))
    full = np.concatenate(outs, axis=0)                  # [8192, 2048]
    return full.reshape(BATCH, SEQ, EMB)
